# revision 11
# baseline (speedup 1.0000x reference)
"""Trainium2 Bass kernel for the DMIL/PCL detection loss (nms_detection).

Contract: kernel(cls_prob[500000,21] f32, boxes[500000,4] f32,
im_labels[1,20] i32) -> scalar f32 loss, matching the jax reference to
within fp32 tolerance.

Strategy (8 NeuronCores, SPMD), v2 — batched big-AP rewrite of the
per-class-loop baseline (216us).  Per-instruction overhead (~280ns)
dominated the baseline, so every per-class loop is replaced with one
instruction over a [128, NP*K] (or [128, 2*NP*K] / [128, 4*NP*K])
access pattern with stride-0 broadcast axes:

  - Shard the N=500000 proposal axis across 8 cores (62500 rows each,
    padded to 63488 = 128 partitions x 496 rows), class-major planes.
  - Phase A: one chunked per-class max reduce; winner box extracted in
    3 batched ops (f32 eq mask, mask*coords, reduce); cross-partition
    via gpsimd partition_all_reduce; one AllGather exchanges
    (score, box); every core selects the global winner per class.
  - Phase B: batched IoU in log space: U/V/W/relu/inter in 5 ops over
    all classes at once, ACT Ln for log terms (per-class Ln with
    per-partition gt-area bias folds the denominator add for free).
  - Phase C: batched win masks + per-class segmented reduces, TensorE
    ones-matmul column sums, one AllReduce, short scalar tail.
"""

import os
import sys
from contextlib import ExitStack

import numpy as np

sys.path.insert(0, "/opt/trn_rl_repo")

NCORES = 8
N = 500000
C = 20
PERCORE = N // NCORES          # 62500
K = 496                        # rows per partition
ROWS = 128 * K                 # 63488 rows per core after padding
INV_N = 1.0 / N
LN13 = float(np.float32(np.log(1.0 / 3.0)))    # ov >= 0.5  <=>  z >= ln(1/3)
LN111 = float(np.float32(np.log(1.0 / 11.0)))  # ov >= 0.1  <=>  z >= ln(1/11)


def _build(present, dbg=False, stage=6):
    import concourse.bacc as bacc
    import concourse.bass_isa as bass_isa
    import concourse.mybir as mybir
    from concourse import tile

    f32 = mybir.dt.float32
    f16 = mybir.dt.float16
    Alu = mybir.AluOpType
    Act = mybir.ActivationFunctionType
    AX = mybir.AxisListType

    NP = len(present)
    NPK = NP * K

    nc = bacc.Bacc("TRN2", target_bir_lowering=False, debug=False,
                   num_devices=NCORES)
    pin = nc.dram_tensor("p", [128, NPK], f32, kind="ExternalInput")
    b16_d = nc.dram_tensor("b16", [128, 6 * K], f16, kind="ExternalInput")
    p16_d = nc.dram_tensor("p16", [128, NPK], f16, kind="ExternalInput")
    loss_out = nc.dram_tensor("loss", [1, 1], f32, kind="ExternalOutput")
    if dbg:
        dbg_a = nc.dram_tensor("dbg_a", [1, 5 * NP], f32, kind="ExternalOutput")
        dbg_g = nc.dram_tensor("dbg_g", [1, 5 * NP], f32, kind="ExternalOutput")
        dbg_f = nc.dram_tensor("dbg_f", [NP, 3], f32, kind="ExternalOutput")

    ctx = ExitStack()
    with ctx:
        tc = ctx.enter_context(tile.TileContext(nc))
        sb = ctx.enter_context(tc.tile_pool(name="sb", bufs=1))
        psum = ctx.enter_context(tc.tile_pool(name="psum", bufs=2, space="PSUM"))
        dram = ctx.enter_context(tc.tile_pool(name="dram", bufs=1, space="DRAM"))
        # rotating scratch rings: 3 live [128,2NPK] + 3 live [128,NPK] f16
        big2 = ctx.enter_context(tc.tile_pool(name="big2", bufs=3))
        big1 = ctx.enter_context(tc.tile_pool(name="big1", bufs=3))

        _cnt = [0]

        def b2t():
            _cnt[0] += 1
            return big2.tile([128, 2 * NPK], f16, tag="b2",
                             name=f"b2_{_cnt[0]}")

        def b1t():
            _cnt[0] += 1
            return big1.tile([128, NPK], f16, tag="b1",
                             name=f"b1_{_cnt[0]}")

        # ---------------- input loads (contiguous, class/coord-major) --------
        P, P_free = tc.tile([128, NPK], f32, name="P")
        bnds = [0, NP // 4, NP // 2, 3 * NP // 4, NP]
        for a, b2 in zip(bnds[:-1], bnds[1:]):
            nc.sync.dma_start(out=P[:, a * K:b2 * K], in_=pin[:, a * K:b2 * K])

        B16 = sb.tile([128, 6 * K], f16, tag="B16")
        nc.sync.dma_start(out=B16[:], in_=b16_d[:, :])
        HXY1 = B16[:, 0:2 * K]        # x1 | y1   (scaled by SCL)
        HXY2 = B16[:, 2 * K:4 * K]    # (x2+1)*SCL | (y2+1)*SCL
        HAB = B16[:, 4 * K:5 * K]     # area_b * SCL^2
        HL0 = B16[:, 5 * K:6 * K]     # ln(clip(p0))
        P16 = sb.tile([128, NPK], f16, tag="P16")
        nc.sync.dma_start(out=P16[:], in_=p16_d[:, :])

        # ---------------- phase A: per-class max + winner box ----------------
        M1 = sb.tile([128, NP], f32, tag="M1")
        for a, b2 in zip(bnds[:-1], bnds[1:]):
            nc.vector.tensor_reduce(
                out=M1[:, a:b2],
                in_=P[:, a * K:b2 * K].rearrange("p (j k) -> p j k", k=K),
                axis=AX.X, op=Alu.max)
        LM = sb.tile([128, NP], f32, tag="LM")
        nc.gpsimd.partition_all_reduce(LM[:], M1[:], channels=128,
                                       reduce_op=bass_isa.ReduceOp.max)

        # batched winner mask + coordinate extraction
        EQ = b1t()
        LMb = LM[:].rearrange("p (j o) -> p j o", o=1).broadcast_to(
            (128, NP, K))
        nc.vector.tensor_tensor(
            out=EQ[:].rearrange("p (j k) -> p j k", k=K),
            in0=P[:].rearrange("p (j k) -> p j k", k=K),
            in1=LMb, op=Alu.is_equal)
        P_free()

        BOXR = sb.tile([128, 4 * NP], f32, tag="BOXR")
        # interleaved [j*4 + d] layout (matches the AllGather DMA pattern)
        BOXRv = BOXR[:].rearrange("p (j d) -> p d j", d=4)
        EQb = EQ[:].rearrange("p (o j k) -> p o j k", k=K, o=1).broadcast_to(
            (128, 2, NP, K))
        for h, hsrc in enumerate((HXY1, HXY2)):
            CRD = b2t()
            HB = hsrc.rearrange("p (c o k) -> p c o k", k=K, o=1).broadcast_to(
                (128, 2, NP, K))
            nc.vector.tensor_tensor(
                out=CRD[:].rearrange("p (c j k) -> p c j k", k=K, j=NP),
                in0=EQb, in1=HB, op=Alu.mult)
            nc.vector.tensor_reduce(
                out=BOXRv[:, 2 * h:2 * h + 2, :],
                in_=CRD[:].rearrange("p (c j k) -> p c j k", k=K, j=NP),
                axis=AX.X, op=Alu.add)
        BOXM = sb.tile([128, 4 * NP], f32, tag="BOXM")
        nc.gpsimd.partition_all_reduce(BOXM[:], BOXR[:], channels=128,
                                       reduce_op=bass_isa.ReduceOp.max)

        # collective input from replicated row 0: [scores | x1 y1 x2 y2 planes]
        t1 = dram.tile([1, 5 * NP], f32)
        nc.sync.dma_start(out=t1[0:1, 0:NP], in_=LM[0:1, :])
        nc.sync.dma_start(out=t1[0:1, NP:5 * NP], in_=BOXM[0:1, :])
        if dbg:
            nc.sync.dma_start(out=dbg_a[:, :], in_=t1[:, :])

        if stage >= 4:
            # exchange (score, box) across cores; select global winner
            ccout = dram.tile([NCORES, 1, 5 * NP], f32)
            nc.gpsimd.collective_compute(
                "AllGather", Alu.bypass,
                replica_groups=[list(range(NCORES))],
                ins=[t1[:].opt()], outs=[ccout[:].opt()])
            XG = sb.tile([NP, NCORES * 5], f32, tag="XG")
            XGvw = XG[:].rearrange("p (r d) -> p r d", d=5)
            nc.sync.dma_start(
                out=XGvw[:, :, 0:1],
                in_=ccout[:, :, 0:NP].rearrange("r o p -> p r o"))
            # boxes arrive interleaved [j*4+d]; transpose to per-class rows
            nc.sync.dma_start(
                out=XGvw[:, :, 1:5],
                in_=ccout[:, :, NP:5 * NP].rearrange(
                    "r o (p d) -> p r (o d)", d=4))
            XGv = XG[:].rearrange("p (r d) -> p r d", d=5)

            gmax = sb.tile([NP, 1], f32, tag="gmax")
            nc.vector.tensor_reduce(out=gmax[:], in_=XGv[:, :, 0], axis=AX.X,
                                    op=Alu.max)
            eq8 = sb.tile([NP, NCORES], f32, tag="eq8")
            nc.vector.tensor_tensor(out=eq8[:], in0=XGv[:, :, 0],
                                    in1=gmax[:].broadcast_to((NP, NCORES)),
                                    op=Alu.is_equal)
            # all 4 coords in one masked op + one reduce
            J8 = sb.tile([NP, 4 * NCORES], f32, tag="J8")
            eq8b = eq8[:].rearrange("p (o r) -> p o r", o=1).broadcast_to(
                (NP, 4, NCORES))
            nc.vector.tensor_tensor(
                out=J8[:].rearrange("p (d r) -> p d r", r=NCORES),
                in0=eq8b, in1=XGv[:, :, 1:5].rearrange("p r d -> p d r"),
                op=Alu.mult)
            GTB = sb.tile([NP, 4], f32, tag="GTB")
            nc.vector.tensor_reduce(
                out=GTB[:], in_=J8[:].rearrange("p (d r) -> p d r", r=NCORES),
                axis=AX.X, op=Alu.max)

            # gt area (scaled), per class, before broadcast
            dg = sb.tile([NP, 2], f32, tag="dg")
            nc.vector.tensor_tensor(out=dg[:], in0=GTB[:, 2:4],
                                    in1=GTB[:, 0:2], op=Alu.subtract)
            AGv = sb.tile([NP, 1], f32, tag="AGv")
            nc.vector.tensor_tensor(out=AGv[:], in0=dg[:, 0:1],
                                    in1=dg[:, 1:2], op=Alu.mult)

            # broadcast gt constants to all partitions via a K=1 matmul
            t2 = dram.tile([NP, 5], f32)
            nc.sync.dma_start(out=t2[:, 0:4], in_=GTB[:])
            nc.sync.dma_start(out=t2[:, 4:5], in_=AGv[:])
            RW = sb.tile([1, 5 * NP], f32, tag="RW")
            nc.sync.dma_start(out=RW[:].rearrange("o (d p) -> o d p", p=NP),
                              in_=t2[:, :].rearrange("(o p) d -> o d p", o=1))
            ones1 = sb.tile([1, 128], f32, tag="ones1")
            nc.vector.memset(ones1[:], 1.0)
            PSg = psum.tile([128, 5 * NP], f32, tag="PSg")
            nc.tensor.matmul(out=PSg[:], lhsT=ones1[:], rhs=RW[:],
                             start=True, stop=True)
            GCONh = sb.tile([128, 4 * NP], f16, tag="GCONh")
            nc.scalar.copy(GCONh[:], PSg[:, 0:4 * NP])
            AGs = sb.tile([128, NP], f32, tag="AGs")
            nc.scalar.copy(AGs[:], PSg[:, 4 * NP:5 * NP])
            if dbg:
                GCf = sb.tile([1, 4 * NP], f32, tag="GCf")
                nc.scalar.copy(GCf[:], GCONh[0:1, :])
                nc.sync.dma_start(out=dbg_g[0:1, 0:4 * NP], in_=GCf[:])
                nc.sync.dma_start(out=dbg_g[0:1, 4 * NP:], in_=AGs[0:1, :])

        if stage >= 5:
            # -------- phase B: batched IoU in log space ----------
            U = b2t()
            V = b2t()

            def hb(src):
                return src.rearrange("p (c o k) -> p c o k", k=K,
                                     o=1).broadcast_to((128, 2, NP, K))

            def gb(src):
                return src.rearrange("p (c j o) -> p c j o", j=NP,
                                     o=1).broadcast_to((128, 2, NP, K))

            def v4(t):
                return t.rearrange("p (c j k) -> p c j k", k=K, j=NP)

            nc.vector.tensor_tensor(out=v4(U[:]), in0=hb(HXY1),
                                    in1=gb(GCONh[:, 0:2 * NP]), op=Alu.max)
            nc.vector.tensor_tensor(out=v4(V[:]), in0=hb(HXY2),
                                    in1=gb(GCONh[:, 2 * NP:4 * NP]),
                                    op=Alu.min)
            W = b2t()
            nc.vector.tensor_tensor(out=W[:], in0=V[:], in1=U[:],
                                    op=Alu.subtract)
            WR = b2t()
            nc.vector.tensor_single_scalar(out=WR[:], in_=W[:], scalar=0.0,
                                           op=Alu.max)
            INTER = b1t()
            nc.vector.tensor_tensor(out=INTER[:], in0=WR[:, 0:NPK],
                                    in1=WR[:, NPK:2 * NPK], op=Alu.mult)

            # log terms on ACT: la_j = ln(area_b + area_g_j) via bias fold
            LAt = b1t()
            for j in range(NP):
                nc.scalar.activation(LAt[:, j * K:(j + 1) * K], HAB,
                                     Act.Ln, bias=AGs[:, j:j + 1])
            LIt = b1t()
            half = (NP // 2) * K
            nc.scalar.activation(LIt[:, 0:half], INTER[:, 0:half], Act.Ln)
            nc.scalar.activation(LIt[:, half:], INTER[:, half:], Act.Ln)
            ZL = sb.tile([128, NPK], f16, tag="ZL")
            nc.vector.tensor_tensor(out=ZL[:], in0=LIt[:], in1=LAt[:],
                                    op=Alu.subtract)

            # RM = max over classes (contiguous pairwise tree)
            TA = b1t()
            TB = b1t()
            n = NP
            if n % 2:
                nc.vector.tensor_copy(TA[:, (n // 2) * K:(n // 2 + 1) * K],
                                      ZL[:, (n - 1) * K:n * K])
            h = n // 2
            nc.vector.tensor_tensor(out=TA[:, 0:h * K], in0=ZL[:, 0:h * K],
                                    in1=ZL[:, h * K:2 * h * K], op=Alu.max)
            n = (n + 1) // 2
            cur, other = TA, TB
            while n > 1:
                h = n // 2
                if n % 2:
                    nc.vector.tensor_copy(other[:, h * K:(h + 1) * K],
                                          cur[:, (n - 1) * K:n * K])
                nc.vector.tensor_tensor(out=other[:, 0:h * K],
                                        in0=cur[:, 0:h * K],
                                        in1=cur[:, h * K:2 * h * K],
                                        op=Alu.max)
                n = (n + 1) // 2
                cur, other = other, cur
            RM = sb.tile([128, K], f16, tag="RM")
            nc.vector.tensor_copy(RM[:], cur[:, 0:K])

            # ---------------- phase C: batched accumulations -------------
            RMf = sb.tile([128, K], f16, tag="RMf")
            nc.vector.tensor_single_scalar(out=RMf[:], in_=RM[:],
                                           scalar=LN13, op=Alu.max)
            fgm = sb.tile([128, K], f16, tag="fgm")
            nc.vector.tensor_single_scalar(out=fgm[:], in_=RM[:],
                                           scalar=LN13, op=Alu.is_ge)
            bgw = sb.tile([128, K], f16, tag="bgw")
            nc.vector.tensor_single_scalar(out=bgw[:], in_=RM[:],
                                           scalar=LN111, op=Alu.is_ge)
            bib = sb.tile([128, K], f16, tag="bib")
            nc.vector.tensor_sub(bib[:], bgw[:], fgm[:])
            base = sb.tile([128, K], f16, tag="base")
            nc.vector.tensor_mul(base[:], HL0, bib[:])

            def kb(src):  # [128,K] -> [128,NP,K] broadcast over classes
                return src.rearrange("p (o k) -> p o k", o=1).broadcast_to(
                    (128, NP, K))

            def zv(t):
                return t.rearrange("p (j k) -> p j k", k=K)

            ACCS = sb.tile([128, 3 * NP], f32, tag="ACCS")
            EQF = b1t()
            nc.vector.tensor_tensor(out=zv(EQF[:]), in0=zv(ZL[:]),
                                    in1=kb(RMf[:]), op=Alu.is_ge)
            nc.vector.tensor_reduce(out=ACCS[:, 0:NP], in_=zv(EQF[:]),
                                    axis=AX.X, op=Alu.add)
            SPW = b1t()
            nc.vector.tensor_tensor(out=SPW[:], in0=EQF[:], in1=P16[:],
                                    op=Alu.mult)
            nc.vector.tensor_reduce(out=ACCS[:, NP:2 * NP], in_=zv(SPW[:]),
                                    axis=AX.X, op=Alu.add)
            EWB = b1t()
            nc.vector.tensor_tensor(out=zv(EWB[:]), in0=zv(ZL[:]),
                                    in1=kb(RM[:]), op=Alu.is_ge)
            NGW = b1t()
            nc.vector.tensor_tensor(out=zv(NGW[:]), in0=zv(EWB[:]),
                                    in1=kb(base[:]), op=Alu.mult)
            nc.vector.tensor_reduce(out=ACCS[:, 2 * NP:3 * NP],
                                    in_=zv(NGW[:]), axis=AX.X, op=Alu.add)

            ones128 = sb.tile([128, 1], f32, tag="ones128")
            nc.vector.memset(ones128[:], 1.0)
            SUMP = psum.tile([3 * NP, 1], f32, tag="SUMP")
            nc.tensor.matmul(out=SUMP[:], lhsT=ACCS[:], rhs=ones128[:],
                             start=True, stop=True)
            SUMS = sb.tile([3 * NP, 1], f32, tag="SUMS")
            nc.scalar.copy(SUMS[:], SUMP[:])

        if stage >= 6:
            cc2in = dram.tile([3 * NP, 1], f32)
            nc.sync.dma_start(out=cc2in[:], in_=SUMS[:])
            cc2out = dram.tile([3 * NP, 1], f32)
            nc.gpsimd.collective_compute(
                "AllReduce", Alu.add,
                replica_groups=[list(range(NCORES))],
                ins=[cc2in[:].opt()], outs=[cc2out[:].opt()])

            FIN = sb.tile([NP, 3], f32, tag="FIN")
            nc.sync.dma_start(out=FIN[:].rearrange("p (d o) -> p d o", o=1),
                              in_=cc2out[:, :].rearrange("(d p) o -> p d o", d=3))
            cntv = FIN[:, 0:1]
            spv = FIN[:, 1:2]
            ngv = FIN[:, 2:3]

            mx = sb.tile([NP, 1], f32, tag="mx")
            nc.vector.tensor_single_scalar(out=mx[:], in_=cntv, scalar=1.0,
                                           op=Alu.max)
            rcv = sb.tile([NP, 1], f32, tag="rcv")
            nc.vector.reciprocal(rcv[:], mx[:])
            mean = sb.tile([NP, 1], f32, tag="mean")
            nc.vector.tensor_mul(mean[:], spv, rcv[:])
            cg = sb.tile([NP, 1], f32, tag="cg")
            nc.vector.tensor_single_scalar(out=cg[:], in_=cntv, scalar=0.5,
                                           op=Alu.is_ge)
            # t = mean - cg;  lnm = ln(t + 1) == ln(mean) when cnt>0 else 0
            tt = sb.tile([NP, 1], f32, tag="tt")
            nc.vector.scalar_tensor_tensor(out=tt[:], in0=cg[:], scalar=-1.0,
                                           in1=mean[:], op0=Alu.mult,
                                           op1=Alu.add)
            lnm = sb.tile([NP, 1], f32, tag="lnm")
            nc.scalar.activation(lnm[:], tt[:], Act.Ln, bias=1.0)
            pv = sb.tile([NP, 1], f32, tag="pv")
            nc.vector.tensor_scalar(out=pv[:], in0=lnm[:], scalar1=cntv,
                                    scalar2=gmax[:], op0=Alu.mult,
                                    op1=Alu.mult)
            nv = sb.tile([NP, 1], f32, tag="nv")
            nc.vector.tensor_mul(nv[:], ngv, gmax[:])
            tot = sb.tile([NP, 1], f32, tag="tot")
            nc.vector.tensor_tensor(out=tot[:], in0=pv[:], in1=nv[:],
                                    op=Alu.add)
            onesNP = sb.tile([NP, 1], f32, tag="onesNP")
            nc.vector.memset(onesNP[:], 1.0)
            LPS = psum.tile([1, 1], f32, tag="LPS")
            nc.tensor.matmul(out=LPS[:], lhsT=tot[:], rhs=onesNP[:],
                             start=True, stop=True)
            LS = sb.tile([1, 1], f32, tag="LS")
            nc.scalar.activation(LS[:], LPS[:], Act.Copy, scale=-INV_N)
            nc.sync.dma_start(out=loss_out[:, :], in_=LS[:])
            if dbg:
                nc.sync.dma_start(out=dbg_f[:, :], in_=FIN[:])
        else:
            LS = sb.tile([1, 1], f32, tag="LS")
            nc.vector.memset(LS[:], 0.0)
            nc.sync.dma_start(out=loss_out[:, :], in_=LS[:])
            if dbg:
                if stage >= 5:
                    nc.sync.dma_start(
                        out=dbg_f[:, :].rearrange("p d -> (d p) 1"), in_=SUMS[:])
                else:
                    Z3 = sb.tile([NP, 3], f32, tag="Z3")
                    nc.vector.memset(Z3[:], 0.0)
                    nc.sync.dma_start(out=dbg_f[:, :], in_=Z3[:])
                if stage < 4:
                    ZG = sb.tile([1, 5 * NP], f32, tag="ZG")
                    nc.vector.memset(ZG[:], 0.0)
                    nc.sync.dma_start(out=dbg_g[:, :], in_=ZG[:])

    nc.compile()
    return nc


def _shard_inputs(cls_prob, boxes, im_labels):
    cls_prob = np.ascontiguousarray(cls_prob, dtype=np.float32)
    boxes = np.ascontiguousarray(boxes, dtype=np.float32)
    presort = np.nonzero(np.asarray(im_labels)[0] > 0)[0]
    NPRES = len(presort)
    in_maps = []
    for core in range(NCORES):
        lo = core * PERCORE
        hi = lo + PERCORE
        p = np.zeros((ROWS, C + 1), dtype=np.float32)
        p[:PERCORE] = cls_prob[lo:hi]
        p[PERCORE:, 0] = 1.0                      # pad: ln(p0)=0, never argmax
        pp = np.zeros((ROWS, NPRES), dtype=np.float32)
        pp[:PERCORE] = cls_prob[lo:hi][:, presort + 1]
        b = np.empty((ROWS, 4), dtype=np.float32)
        b[:PERCORE] = boxes[lo:hi]
        b[PERCORE:] = [-20000.0, -20000.0, -19999.0, -19999.0]   # zero-IoU pad
        # class-major / coord-major: [128, NP, 496] and [128, 6, 496]
        pcm = np.ascontiguousarray(
            pp.reshape(128, K, NPRES).transpose(0, 2, 1)).reshape(128, NPRES * K)
        SCL = np.float32(0.25)
        h = np.empty((ROWS, 6), dtype=np.float32)
        h[:, 0] = b[:, 0] * SCL
        h[:, 1] = b[:, 1] * SCL
        h[:, 2] = (b[:, 2] + 1.0) * SCL
        h[:, 3] = (b[:, 3] + 1.0) * SCL
        h[:, 4] = ((b[:, 2] - b[:, 0] + 1.0) * (b[:, 3] - b[:, 1] + 1.0)
                   * SCL * SCL)
        h[:, 5] = np.log(np.clip(p[:, 0], 1e-9, 1.0 - 1e-9))
        hcm = np.ascontiguousarray(
            h.reshape(128, K, 6).transpose(0, 2, 1)).reshape(
                128, 6 * K).astype(np.float16)
        in_maps.append({"p": pcm, "b16": hcm,
                        "p16": pcm.astype(np.float16)})
    return in_maps


_CACHE = {}


def kernel(cls_prob, boxes, im_labels, _trace=False, _dbg=False, _stage=6):
    from concourse.bass_utils import run_bass_kernel_spmd

    present = tuple(int(c) for c in np.nonzero(np.asarray(im_labels)[0] > 0)[0])
    key = (present, _dbg, _stage)
    if key not in _CACHE:
        _CACHE[key] = _build(present, dbg=_dbg, stage=_stage)
    nc = _CACHE[key]

    in_maps = _shard_inputs(cls_prob, boxes, im_labels)
    res = run_bass_kernel_spmd(nc, in_maps, list(range(NCORES)), trace=_trace)
    out = np.float32(res.results[0]["loss"][0, 0])
    if _trace or _dbg:
        kernel._last = res
    return np.asarray(out)


if __name__ == "__main__":
    cls_prob = np.load("/tmp/cls_prob.npy")
    boxes = np.load("/tmp/boxes.npy")
    im_labels = np.load("/tmp/im_labels.npy")
    stage = int(os.environ.get("KSTAGE", "6"))
    dbg = os.environ.get("KDBG") == "1"
    out = kernel(cls_prob, boxes, im_labels, _dbg=dbg, _stage=stage)
    print("kernel loss:", out)
    if dbg and hasattr(kernel, "_last"):
        r0 = kernel._last.results[0]
        for kk in ("dbg_a", "dbg_g", "dbg_f"):
            if kk in r0:
                print(kk, np.array2string(r0[kk], precision=4, suppress_small=False))


# revision 27
# speedup vs baseline: 1.3444x; 1.3444x over previous
"""Trainium2 Bass kernel for the DMIL/PCL detection loss (nms_detection).

Contract: kernel(cls_prob[500000,21] f32, boxes[500000,4] f32,
im_labels[1,20] i32) -> scalar f32 loss, matching the jax reference to
within fp32 tolerance.

Strategy (8 NeuronCores, SPMD), v4.  DVE perf modes measured on HW:
tensor_scalar/scalar_tensor_tensor hit 4x (0.27ns/elem, per-partition
scalars exempt from operand checks), tensor_tensor hits 2x only with
all-2-byte stride-1-innermost operands, tensor_reduce is always 1x.
So: every per-class computation is a K-wide STT with the per-class
constant in the scalar slot and the per-class sum fused via accum_out;
no big reduces anywhere.

  - Shard N=500000 proposals across 8 cores (62500 rows, padded to
    63488 = 128 partitions x 496), class-major planes.
  - Phase A: chunked per-class max; winner located by one fused STT per
    class ((P==max)*iota, accum -> index); box coords fetched by a
    16-row dma_gather from a raw per-core box table; one AllGather of
    (score, box); global winner selected on every core.
  - Phase B: per-class TSS/STT IoU (U then fused min/sub), batched
    relu/inter, ACT Ln with per-partition bias for the log terms.
  - Phase C: per-class STT win masks with counts/prob-sums/bg-sums
    fused into accum_out, TensorE ones-matmul column sums, one
    AllReduce, short scalar tail.
"""

import os
import sys
from contextlib import ExitStack

import numpy as np

sys.path.insert(0, "/opt/trn_rl_repo")

NCORES = 8
N = 500000
C = 20
PERCORE = N // NCORES          # 62500
K = 496                        # rows per partition
ROWS = 128 * K                 # 63488 rows per core after padding
INV_N = 1.0 / N
LN13 = float(np.float32(np.log(1.0 / 3.0)))    # ov >= 0.5  <=>  z >= ln(1/3)
LN111 = float(np.float32(np.log(1.0 / 11.0)))  # ov >= 0.1  <=>  z >= ln(1/11)


def _build(present, dbg=False, stage=6):
    import concourse.bacc as bacc
    import concourse.bass_isa as bass_isa
    import concourse.mybir as mybir
    from concourse import tile

    f32 = mybir.dt.float32
    f16 = mybir.dt.float16
    i16 = mybir.dt.int16
    i32 = mybir.dt.int32
    Alu = mybir.AluOpType
    Act = mybir.ActivationFunctionType
    AX = mybir.AxisListType

    NP = len(present)
    NPK = NP * K

    nc = bacc.Bacc("TRN2", target_bir_lowering=False, debug=False,
                   num_devices=NCORES)
    pin = nc.dram_tensor("p", [128, NPK], f32, kind="ExternalInput")
    b16_d = nc.dram_tensor("b16", [128, 7 * K], f16, kind="ExternalInput")
    p16_d = nc.dram_tensor("p16", [128, NPK], f16, kind="ExternalInput")
    poff_d = nc.dram_tensor("poff", [128, 2], f32, kind="ExternalInput")
    braw_d = nc.dram_tensor("braw", [ROWS // 16, 64], f32, kind="ExternalInput")
    loss_out = nc.dram_tensor("loss", [1, 1], f32, kind="ExternalOutput")
    if dbg:
        dbg_a = nc.dram_tensor("dbg_a", [1, 5 * NP], f32, kind="ExternalOutput")
        dbg_g = nc.dram_tensor("dbg_g", [1, 16 * NP], f32, kind="ExternalOutput")
        dbg_f = nc.dram_tensor("dbg_f", [NP, 3], f32, kind="ExternalOutput")

    ctx = ExitStack()
    with ctx:
        tc = ctx.enter_context(tile.TileContext(nc))
        sb = ctx.enter_context(tc.tile_pool(name="sb", bufs=1))
        psum = ctx.enter_context(tc.tile_pool(name="psum", bufs=2, space="PSUM"))
        dram = ctx.enter_context(tc.tile_pool(name="dram", bufs=1, space="DRAM"))

        # ---------------- input loads (contiguous, class/coord-major) --------
        P = sb.tile([128, NPK], f32, tag="P")
        bnds = [0, NP // 4, NP // 2, 3 * NP // 4, NP]
        for a, b2 in zip(bnds[:-1], bnds[1:]):
            nc.sync.dma_start(out=P[:, a * K:b2 * K], in_=pin[:, a * K:b2 * K])

        B16 = sb.tile([128, 7 * K], f16, tag="B16")
        nc.sync.dma_start(out=B16[:], in_=b16_d[:, :])
        HX1 = B16[:, 0:K]             # x1*SCL
        HY1 = B16[:, K:2 * K]         # y1*SCL
        HX2 = B16[:, 2 * K:3 * K]     # (x2+1)*SCL
        HY2 = B16[:, 3 * K:4 * K]     # (y2+1)*SCL
        HAB = B16[:, 4 * K:5 * K]     # area_b * SCL^2
        HL0 = B16[:, 5 * K:6 * K]     # ln(clip(p0))
        IOT = B16[:, 6 * K:7 * K]     # k+1 (1..496)
        POFF2 = sb.tile([128, 2], f32, tag="POFF2")
        nc.sync.dma_start(out=POFF2[:], in_=poff_d[:, :])
        POFF = POFF2[:, 0:1]          # p*496
        PMOD = POFF2[:, 1:2]          # (p%16)+1
        P16 = sb.tile([128, NPK], f16, tag="P16")
        nc.sync.dma_start(out=P16[:], in_=p16_d[:, :])

        def Pp(j):
            return P[:, j * K:(j + 1) * K]

        # ---------------- phase A: per-class max + winner flat index ---------
        M1 = sb.tile([128, NP], f32, tag="M1")
        for a, b2 in zip(bnds[:-1], bnds[1:]):
            nc.vector.tensor_reduce(
                out=M1[:, a:b2],
                in_=P[:, a * K:b2 * K].rearrange("p (j k) -> p j k", k=K),
                axis=AX.X, op=Alu.max)
        LM = sb.tile([128, NP], f32, tag="LM")
        nc.gpsimd.partition_all_reduce(LM[:], M1[:], channels=128,
                                       reduce_op=bass_isa.ReduceOp.max)

        # fused (P == max) * (k+1), accum -> per-partition winner index
        KIr = sb.tile([128, NP], f32, tag="KIr")
        junkA = sb.tile([128, K], f16, tag="junkA")
        for j in range(NP):
            nc.vector.scalar_tensor_tensor(
                out=junkA[:], in0=Pp(j), scalar=LM[:, j:j + 1], in1=IOT,
                op0=Alu.is_equal, op1=Alu.mult,
                accum_out=KIr[:, j:j + 1])

        wp = sb.tile([128, NP], f32, tag="wp")
        nc.vector.tensor_tensor(out=wp[:], in0=M1[:], in1=LM[:],
                                op=Alu.is_equal)
        FL1 = sb.tile([128, NP], f32, tag="FL1")
        nc.vector.tensor_tensor(out=FL1[:], in0=KIr[:],
                                in1=POFF[:].broadcast_to((128, NP)),
                                op=Alu.add)
        nc.vector.tensor_tensor(out=FL1[:], in0=FL1[:], in1=wp[:],
                                op=Alu.mult)
        FLG = sb.tile([128, NP], f32, tag="FLG")
        nc.gpsimd.partition_all_reduce(FLG[:], FL1[:], channels=128,
                                       reduce_op=bass_isa.ReduceOp.max)
        # FLG row: p*496 + k* + 1 per class, replicated on all partitions

        # per-partition pick of class (p%16): idx pattern must repeat in all
        # eight 16-partition groups (one per gpsimd core)
        OHp = sb.tile([128, NP], f16, tag="OHp")
        nc.vector.tensor_single_scalar(out=OHp[:], in_=IOT[:, 0:NP],
                                       scalar=PMOD[:, 0:1], op=Alu.is_equal)
        junkP = sb.tile([128, NP], f16, tag="junkP")
        FLPa = sb.tile([128, 1], f32, tag="FLPa")
        nc.vector.scalar_tensor_tensor(out=junkP[:], in0=OHp[:], scalar=1.0,
                                       in1=FLG[:], op0=Alu.mult, op1=Alu.mult,
                                       accum_out=FLPa[:])
        F0 = sb.tile([128, 1], f32, tag="F0")
        nc.vector.tensor_scalar(out=F0[:], in0=FLPa[:], scalar1=-1.0,
                                scalar2=0.0, op0=Alu.add, op1=Alu.max)
        # chunk row r>>4 (int16-sized) and offset (r&15)+1 within the chunk
        FI = sb.tile([128, 1], i32, tag="FI")
        nc.vector.tensor_copy(FI[:], F0[:])
        OFFi = sb.tile([128, 1], i32, tag="OFFi")
        nc.vector.tensor_scalar(out=OFFi[:], in0=FI[:], scalar1=15,
                                scalar2=None, op0=Alu.bitwise_and)
        PRi = sb.tile([128, 1], i32, tag="PRi")
        nc.vector.tensor_scalar(out=PRi[:], in0=FI[:], scalar1=4,
                                scalar2=None, op0=Alu.arith_shift_right)
        PRI = sb.tile([128, 1], i16, tag="PRI")
        nc.vector.tensor_copy(PRI[:], PRi[:])
        OFF1 = sb.tile([128, 1], f32, tag="OFF1")
        nc.vector.tensor_copy(OFF1[:], OFFi[:])
        nc.vector.tensor_scalar(out=OFF1[:], in0=OFF1[:], scalar1=1.0,
                                scalar2=None, op0=Alu.add)

        # gather the winner 16-box chunks (64 f32 = 256B each)
        G64 = sb.tile([128, 64], f32, tag="G64")
        nc.gpsimd.dma_gather(
            out_ap=G64[:].rearrange("p (o d) -> p o d", o=1),
            in_ap=braw_d[:, :], idxs_ap=PRI[:], num_idxs=16,
            num_idxs_reg=16, elem_size=64)
        # one-hot select of box (r&15) inside the chunk: IOT cols 0..15 hold
        # values 1..16 on every partition
        OH = sb.tile([16, 16], f16, tag="OH")
        nc.vector.tensor_single_scalar(out=OH[:], in_=IOT[0:16, 0:16],
                                       scalar=OFF1[0:16, 0:1],
                                       op=Alu.is_equal)
        BOX4 = sb.tile([16, 4], f32, tag="BOX4")
        junkB = sb.tile([16, 16], f32, tag="junkB")
        G64v = G64[:].rearrange("p (i d) -> p i d", d=4)
        for d in range(4):
            nc.vector.scalar_tensor_tensor(
                out=junkB[:], in0=OH[:], scalar=1.0,
                in1=G64v[0:16, :, d], op0=Alu.mult, op1=Alu.mult,
                accum_out=BOX4[:, d:d + 1])

        # collective input: [scores | interleaved (j,d) boxes]
        t1 = dram.tile([1, 5 * NP], f32)
        nc.sync.dma_start(out=t1[0:1, 0:NP], in_=LM[0:1, :])
        nc.sync.dma_start(
            out=t1[0:1, NP:5 * NP].rearrange("o (p d) -> p (o d)", d=4),
            in_=BOX4[0:NP, :])
        if dbg:
            nc.sync.dma_start(out=dbg_a[:, :], in_=t1[:, :])

        if stage >= 4:
            # exchange (score, box) across cores; select global winner
            ccout = dram.tile([NCORES, 1, 5 * NP], f32)
            nc.gpsimd.collective_compute(
                "AllGather", Alu.bypass,
                replica_groups=[list(range(NCORES))],
                ins=[t1[:].opt()], outs=[ccout[:].opt()])
            XG = sb.tile([NP, NCORES * 5], f32, tag="XG")
            XGvw = XG[:].rearrange("p (r d) -> p r d", d=5)
            nc.sync.dma_start(
                out=XGvw[:, :, 0:1],
                in_=ccout[:, :, 0:NP].rearrange("r o p -> p r o"))
            nc.sync.dma_start(
                out=XGvw[:, :, 1:5],
                in_=ccout[:, :, NP:5 * NP].rearrange(
                    "r o (p d) -> p r (o d)", d=4))
            XGv = XG[:].rearrange("p (r d) -> p r d", d=5)

            gmax = sb.tile([NP, 1], f32, tag="gmax")
            nc.vector.tensor_reduce(out=gmax[:], in_=XGv[:, :, 0], axis=AX.X,
                                    op=Alu.max)
            eq8 = sb.tile([NP, NCORES], f32, tag="eq8")
            nc.vector.tensor_tensor(out=eq8[:], in0=XGv[:, :, 0],
                                    in1=gmax[:].broadcast_to((NP, NCORES)),
                                    op=Alu.is_equal)
            J8 = sb.tile([NP, 4 * NCORES], f32, tag="J8")
            eq8b = eq8[:].rearrange("p (o r) -> p o r", o=1).broadcast_to(
                (NP, 4, NCORES))
            nc.vector.tensor_tensor(
                out=J8[:].rearrange("p (d r) -> p d r", r=NCORES),
                in0=eq8b, in1=XGv[:, :, 1:5].rearrange("p r d -> p d r"),
                op=Alu.mult)
            GTB = sb.tile([NP, 4], f32, tag="GTB")
            nc.vector.tensor_reduce(
                out=GTB[:], in_=J8[:].rearrange("p (d r) -> p d r", r=NCORES),
                axis=AX.X, op=Alu.max)

            dg = sb.tile([NP, 2], f32, tag="dg")
            nc.vector.tensor_tensor(out=dg[:], in0=GTB[:, 2:4],
                                    in1=GTB[:, 0:2], op=Alu.subtract)
            AGv = sb.tile([NP, 1], f32, tag="AGv")
            nc.vector.tensor_tensor(out=AGv[:], in0=dg[:, 0:1],
                                    in1=dg[:, 1:2], op=Alu.mult)

            # broadcast gt constants to all partitions via a K=1 matmul
            t2 = dram.tile([NP, 5], f32)
            nc.sync.dma_start(out=t2[:, 0:4], in_=GTB[:])
            nc.sync.dma_start(out=t2[:, 4:5], in_=AGv[:])
            RW = sb.tile([1, 5 * NP], f32, tag="RW")
            nc.sync.dma_start(out=RW[:].rearrange("o (d p) -> o d p", p=NP),
                              in_=t2[:, :].rearrange("(o p) d -> o d p", o=1))
            ones1 = sb.tile([1, 128], f32, tag="ones1")
            nc.vector.memset(ones1[:], 1.0)
            PSg = psum.tile([128, 5 * NP], f32, tag="PSg")
            nc.tensor.matmul(out=PSg[:], lhsT=ones1[:], rhs=RW[:],
                             start=True, stop=True)
            GCON = sb.tile([128, 5 * NP], f32, tag="GCON")
            nc.scalar.copy(GCON[:], PSg[:])
            gx1 = GCON[:, 0 * NP:1 * NP]
            gy1 = GCON[:, 1 * NP:2 * NP]
            gx2 = GCON[:, 2 * NP:3 * NP]
            gy2 = GCON[:, 3 * NP:4 * NP]
            AGs = GCON[:, 4 * NP:5 * NP]
            if dbg:
                nc.sync.dma_start(out=dbg_g[0:1, 0:5 * NP], in_=GCON[0:1, :])

        if stage >= 5:
            # -------- phase B: per-class TSS/STT IoU, log space ----------
            U = sb.tile([128, 2 * NPK], f16, tag="U")
            W = sb.tile([128, 2 * NPK], f16, tag="W")
            for j in range(NP):
                ux = U[:, j * K:(j + 1) * K]
                uy = U[:, (NP + j) * K:(NP + j + 1) * K]
                nc.vector.tensor_single_scalar(out=ux, in_=HX1,
                                               scalar=gx1[:, j:j + 1],
                                               op=Alu.max)
                nc.vector.tensor_single_scalar(out=uy, in_=HY1,
                                               scalar=gy1[:, j:j + 1],
                                               op=Alu.max)
                nc.vector.scalar_tensor_tensor(
                    out=W[:, j * K:(j + 1) * K], in0=HX2,
                    scalar=gx2[:, j:j + 1], in1=ux,
                    op0=Alu.min, op1=Alu.subtract)
                nc.vector.scalar_tensor_tensor(
                    out=W[:, (NP + j) * K:(NP + j + 1) * K], in0=HY2,
                    scalar=gy2[:, j:j + 1], in1=uy,
                    op0=Alu.min, op1=Alu.subtract)
            # WR overwrites U (U is dead once W is built)
            nc.vector.tensor_single_scalar(out=U[:], in_=W[:], scalar=0.0,
                                           op=Alu.max)
            INTER = sb.tile([128, NPK], f16, tag="INTER")
            nc.vector.tensor_tensor(out=INTER[:], in0=U[:, 0:NPK],
                                    in1=U[:, NPK:2 * NPK], op=Alu.mult)

            # log terms on ACT: la_j = ln(area_b + area_g_j) via bias fold
            LAt = sb.tile([128, NPK], f16, tag="LAt")
            for j in range(NP):
                nc.scalar.activation(LAt[:, j * K:(j + 1) * K], HAB,
                                     Act.Ln, bias=AGs[:, j:j + 1])
            LIt = W[:, 0:NPK]        # W is dead once the relu landed in U
            ZL = sb.tile([128, NPK], f16, tag="ZL")
            nql = 4
            qb = [(i * NP // nql) * K for i in range(nql)] + [NPK]
            for i in range(nql):
                nc.scalar.activation(LIt[:, qb[i]:qb[i + 1]],
                                     INTER[:, qb[i]:qb[i + 1]], Act.Ln)
                nc.vector.tensor_tensor(out=ZL[:, qb[i]:qb[i + 1]],
                                        in0=LIt[:, qb[i]:qb[i + 1]],
                                        in1=LAt[:, qb[i]:qb[i + 1]],
                                        op=Alu.subtract)

            # RM = max over classes (contiguous pairwise tree)
            TA = sb.tile([128, 8 * K], f16, tag="TA")
            TB = sb.tile([128, 4 * K], f16, tag="TB")
            n = NP
            if n % 2:
                nc.vector.tensor_copy(TA[:, (n // 2) * K:(n // 2 + 1) * K],
                                      ZL[:, (n - 1) * K:n * K])
            h = n // 2
            nc.vector.tensor_tensor(out=TA[:, 0:h * K], in0=ZL[:, 0:h * K],
                                    in1=ZL[:, h * K:2 * h * K], op=Alu.max)
            n = (n + 1) // 2
            cur, other = TA, TB
            while n > 1:
                h = n // 2
                if n % 2:
                    nc.vector.tensor_copy(other[:, h * K:(h + 1) * K],
                                          cur[:, (n - 1) * K:n * K])
                nc.vector.tensor_tensor(out=other[:, 0:h * K],
                                        in0=cur[:, 0:h * K],
                                        in1=cur[:, h * K:2 * h * K],
                                        op=Alu.max)
                n = (n + 1) // 2
                cur, other = other, cur
            RM = sb.tile([128, K], f16, tag="RM")
            nc.vector.tensor_copy(RM[:], cur[:, 0:K])

            # ---------------- phase C: fused masks + accumulations -------
            RMf = sb.tile([128, K], f16, tag="RMf")
            nc.vector.tensor_single_scalar(out=RMf[:], in_=RM[:],
                                           scalar=LN13, op=Alu.max)
            fgm = sb.tile([128, K], f16, tag="fgm")
            nc.vector.tensor_single_scalar(out=fgm[:], in_=RM[:],
                                           scalar=LN13, op=Alu.is_ge)
            bgw = sb.tile([128, K], f16, tag="bgw")
            nc.vector.tensor_single_scalar(out=bgw[:], in_=RM[:],
                                           scalar=LN111, op=Alu.is_ge)
            bib = sb.tile([128, K], f16, tag="bib")
            nc.vector.tensor_sub(bib[:], bgw[:], fgm[:])
            base = sb.tile([128, K], f16, tag="base")
            nc.vector.tensor_mul(base[:], HL0, bib[:])

            ACCS = sb.tile([128, 3 * NP], f32, tag="ACCS")
            EQF = LAt        # LAt is dead once ZL is built
            junkC = sb.tile([128, K], f16, tag="junkC")
            for j in range(NP):
                zj = ZL[:, j * K:(j + 1) * K]
                nc.vector.scalar_tensor_tensor(
                    out=EQF[:, j * K:(j + 1) * K], in0=zj, scalar=1.0,
                    in1=RMf[:], op0=Alu.mult, op1=Alu.is_ge,
                    accum_out=ACCS[:, j:j + 1])
                nc.vector.scalar_tensor_tensor(
                    out=junkC[:], in0=EQF[:, j * K:(j + 1) * K], scalar=1.0,
                    in1=P16[:, j * K:(j + 1) * K], op0=Alu.mult, op1=Alu.mult,
                    accum_out=ACCS[:, NP + j:NP + j + 1])
            EWB = sb.tile([128, NPK], f16, tag="EWB")
            RMb = RM[:].rearrange("p (o k) -> p o k", o=1).broadcast_to(
                (128, NP, K))
            nc.vector.tensor_tensor(
                out=EWB[:].rearrange("p (j k) -> p j k", k=K),
                in0=ZL[:].rearrange("p (j k) -> p j k", k=K),
                in1=RMb, op=Alu.is_ge)
            for j in range(NP):
                nc.vector.scalar_tensor_tensor(
                    out=junkC[:], in0=EWB[:, j * K:(j + 1) * K], scalar=1.0,
                    in1=base[:], op0=Alu.mult, op1=Alu.mult,
                    accum_out=ACCS[:, 2 * NP + j:2 * NP + j + 1])

            ones128 = sb.tile([128, 1], f32, tag="ones128")
            nc.vector.memset(ones128[:], 1.0)
            SUMP = psum.tile([3 * NP, 1], f32, tag="SUMP")
            nc.tensor.matmul(out=SUMP[:], lhsT=ACCS[:], rhs=ones128[:],
                             start=True, stop=True)
            SUMS = sb.tile([3 * NP, 1], f32, tag="SUMS")
            nc.scalar.copy(SUMS[:], SUMP[:])

        if stage >= 6:
            cc2in = dram.tile([3 * NP, 1], f32)
            nc.sync.dma_start(out=cc2in[:], in_=SUMS[:])
            cc2out = dram.tile([3 * NP, 1], f32)
            nc.gpsimd.collective_compute(
                "AllReduce", Alu.add,
                replica_groups=[list(range(NCORES))],
                ins=[cc2in[:].opt()], outs=[cc2out[:].opt()])

            FIN = sb.tile([NP, 3], f32, tag="FIN")
            nc.sync.dma_start(out=FIN[:].rearrange("p (d o) -> p d o", o=1),
                              in_=cc2out[:, :].rearrange("(d p) o -> p d o", d=3))
            cntv = FIN[:, 0:1]
            spv = FIN[:, 1:2]
            ngv = FIN[:, 2:3]

            mx = sb.tile([NP, 1], f32, tag="mx")
            nc.vector.tensor_single_scalar(out=mx[:], in_=cntv, scalar=1.0,
                                           op=Alu.max)
            rcv = sb.tile([NP, 1], f32, tag="rcv")
            nc.vector.reciprocal(rcv[:], mx[:])
            mean = sb.tile([NP, 1], f32, tag="mean")
            nc.vector.tensor_mul(mean[:], spv, rcv[:])
            cg = sb.tile([NP, 1], f32, tag="cg")
            nc.vector.tensor_single_scalar(out=cg[:], in_=cntv, scalar=0.5,
                                           op=Alu.is_ge)
            # t = mean - cg;  ln(t + 1) == ln(mean) when cnt>0 else 0
            tt2 = sb.tile([NP, 1], f32, tag="tt2")
            nc.vector.scalar_tensor_tensor(out=tt2[:], in0=cg[:], scalar=-1.0,
                                           in1=mean[:], op0=Alu.mult,
                                           op1=Alu.add)
            lnm = sb.tile([NP, 1], f32, tag="lnm")
            nc.scalar.activation(lnm[:], tt2[:], Act.Ln, bias=1.0)
            pv = sb.tile([NP, 1], f32, tag="pv")
            nc.vector.tensor_scalar(out=pv[:], in0=lnm[:], scalar1=cntv,
                                    scalar2=gmax[:], op0=Alu.mult,
                                    op1=Alu.mult)
            nv = sb.tile([NP, 1], f32, tag="nv")
            nc.vector.tensor_mul(nv[:], ngv, gmax[:])
            tot = sb.tile([NP, 1], f32, tag="tot")
            nc.vector.tensor_tensor(out=tot[:], in0=pv[:], in1=nv[:],
                                    op=Alu.add)
            onesNP = sb.tile([NP, 1], f32, tag="onesNP")
            nc.vector.memset(onesNP[:], 1.0)
            LPS = psum.tile([1, 1], f32, tag="LPS")
            nc.tensor.matmul(out=LPS[:], lhsT=tot[:], rhs=onesNP[:],
                             start=True, stop=True)
            LS = sb.tile([1, 1], f32, tag="LS")
            nc.scalar.activation(LS[:], LPS[:], Act.Copy, scale=-INV_N)
            nc.sync.dma_start(out=loss_out[:, :], in_=LS[:])
            if dbg:
                nc.sync.dma_start(out=dbg_f[:, :], in_=FIN[:])
        else:
            LS = sb.tile([1, 1], f32, tag="LS")
            nc.vector.memset(LS[:], 0.0)
            nc.sync.dma_start(out=loss_out[:, :], in_=LS[:])
            if dbg:
                if stage >= 5:
                    nc.sync.dma_start(
                        out=dbg_f[:, :].rearrange("p d -> (d p) 1"), in_=SUMS[:])
                else:
                    Z3 = sb.tile([NP, 3], f32, tag="Z3")
                    nc.vector.memset(Z3[:], 0.0)
                    nc.sync.dma_start(out=dbg_f[:, :], in_=Z3[:])
                if stage < 4:
                    ZG = sb.tile([1, 16 * NP], f32, tag="ZG")
                    nc.vector.memset(ZG[:], 0.0)
                    nc.sync.dma_start(out=dbg_g[:, :], in_=ZG[:])
                    nc.sync.dma_start(out=dbg_g[0:1, 0:NP], in_=FLG[0:1, :])
                    nc.sync.dma_start(
                        out=dbg_g[0:1, NP:2 * NP].rearrange("o p -> p o"),
                        in_=OFF1[0:NP, :])
                    PRIF = sb.tile([128, 1], f32, tag="PRIF")
                    nc.vector.tensor_copy(PRIF[:], PRI[:])
                    nc.sync.dma_start(
                        out=dbg_g[0:1, 2 * NP:3 * NP].rearrange("o p -> p o"),
                        in_=PRIF[0:NP, :])
                    nc.sync.dma_start(
                        out=dbg_g[0:1, 3 * NP:7 * NP].rearrange(
                            "o (p d) -> p o d", d=4),
                        in_=G64[0:NP, 0:4].rearrange("p (o d) -> p o d", o=1))
                    nc.sync.dma_start(
                        out=dbg_g[0:1, 7 * NP:11 * NP].rearrange(
                            "o (p d) -> p o d", d=4),
                        in_=BOX4[0:NP, :].rearrange("p (o d) -> p o d", o=1))

    nc.compile()
    return nc


def _shard_inputs(cls_prob, boxes, im_labels):
    cls_prob = np.ascontiguousarray(cls_prob, dtype=np.float32)
    boxes = np.ascontiguousarray(boxes, dtype=np.float32)
    presort = np.nonzero(np.asarray(im_labels)[0] > 0)[0]
    NPRES = len(presort)
    parts = np.arange(128, dtype=np.float32)
    poff = np.stack([parts * K, (parts % 16) + 1.0], axis=1)
    in_maps = []
    for core in range(NCORES):
        lo = core * PERCORE
        hi = lo + PERCORE
        p = np.zeros((ROWS, C + 1), dtype=np.float32)
        p[:PERCORE] = cls_prob[lo:hi]
        p[PERCORE:, 0] = 1.0                      # pad: ln(p0)=0, never argmax
        pp = np.zeros((ROWS, NPRES), dtype=np.float32)
        pp[:PERCORE] = cls_prob[lo:hi][:, presort + 1]
        b = np.empty((ROWS, 4), dtype=np.float32)
        b[:PERCORE] = boxes[lo:hi]
        b[PERCORE:] = [-20000.0, -20000.0, -19999.0, -19999.0]   # zero-IoU pad
        # class-major planes: [128, NP, 496]
        pcm = np.ascontiguousarray(
            pp.reshape(128, K, NPRES).transpose(0, 2, 1)).reshape(128, NPRES * K)
        SCL = np.float32(0.25)
        h = np.empty((ROWS, 7), dtype=np.float32)
        h[:, 0] = b[:, 0] * SCL
        h[:, 1] = b[:, 1] * SCL
        h[:, 2] = (b[:, 2] + 1.0) * SCL
        h[:, 3] = (b[:, 3] + 1.0) * SCL
        h[:, 4] = ((b[:, 2] - b[:, 0] + 1.0) * (b[:, 3] - b[:, 1] + 1.0)
                   * SCL * SCL)
        h[:, 5] = np.log(np.clip(p[:, 0], 1e-9, 1.0 - 1e-9))
        h[:, 6] = (np.arange(ROWS) % K) + 1.0      # within-partition iota
        hcm = np.ascontiguousarray(
            h.reshape(128, K, 7).transpose(0, 2, 1)).reshape(
                128, 7 * K).astype(np.float16)
        braw = np.ascontiguousarray(h[:, 0:4]).reshape(ROWS // 16, 64)
        in_maps.append({"p": pcm, "b16": hcm, "p16": pcm.astype(np.float16),
                        "poff": poff, "braw": braw})
    return in_maps


_CACHE = {}


def kernel(cls_prob, boxes, im_labels, _trace=False, _dbg=False, _stage=6):
    from concourse.bass_utils import run_bass_kernel_spmd

    present = tuple(int(c) for c in np.nonzero(np.asarray(im_labels)[0] > 0)[0])
    key = (present, _dbg, _stage)
    if key not in _CACHE:
        _CACHE[key] = _build(present, dbg=_dbg, stage=_stage)
    nc = _CACHE[key]

    in_maps = _shard_inputs(cls_prob, boxes, im_labels)
    res = run_bass_kernel_spmd(nc, in_maps, list(range(NCORES)), trace=_trace)
    out = np.float32(res.results[0]["loss"][0, 0])
    if _trace or _dbg:
        kernel._last = res
    return np.asarray(out)


if __name__ == "__main__":
    cls_prob = np.load("/tmp/cls_prob.npy")
    boxes = np.load("/tmp/boxes.npy")
    im_labels = np.load("/tmp/im_labels.npy")
    stage = int(os.environ.get("KSTAGE", "6"))
    dbg = os.environ.get("KDBG") == "1"
    out = kernel(cls_prob, boxes, im_labels, _dbg=dbg, _stage=stage)
    print("kernel loss:", out)
    if dbg and hasattr(kernel, "_last"):
        r0 = kernel._last.results[0]
        for kk in ("dbg_a", "dbg_g", "dbg_f"):
            if kk in r0:
                print(kk, np.array2string(r0[kk], precision=4, suppress_small=False))
